# revision 65
# baseline (speedup 1.0000x reference)
"""Trainium2 Bass kernel for nn_ModelNew_3556232922055 (dense_cnn).

Semantics (per image):
  y8[j]    = conv2d_valid(x, weight[:8]) + bias[:8]          (8,126,126)
  acc[co]  = max over (ci,kh,kw) of 2*W[co,ci,kh,kw]*y8[ci,h+kh,w+kw]
             (out-of-range taps excluded at the bottom/right borders)
  out      = min over co of acc                              (1,126,126)

Sharding: data-parallel over batch, 1 image per NeuronCore (8 cores).

Device mapping per core:
  - host-built im2col X72 [72, 16128] bf16, streamed per conv chunk
  - conv as k=72 bf16 matmuls -> PSUM -> ACT evac (+bias) into
    Y8REP [128, 16128] bf16 where partition p = ci*16 + r holds y8[ci]
  - step 2 in row bands; per band, per tap (kh,kw), per co-half
    (A: co 0-15 on partition ci*16+co, B: co 16-31):
    product = scal[p,t]*y8[ci, pix+off] on DVE (4x ts) or ACT (mul),
    then DVE tensor_tensor(max) into pacc[half]
  - fold of band k emitted during band k+1: PE-transpose 128x128
    chunks -> ACT evac -> DVE max-tree over ci, min over 32 co
    -> OUT[w,h]; finally PE transpose -> DMA out (126,126) f32.
"""

import numpy as np
from contextlib import ExitStack

import concourse.bass as bass
import concourse.tile as tile
from concourse import bacc, mybir
from concourse import masks
from concourse.bass_utils import run_bass_kernel_spmd

try:
    import ml_dtypes
    BF16_NP = ml_dtypes.bfloat16
except ImportError:  # pragma: no cover
    import jax.numpy as jnp
    BF16_NP = jnp.bfloat16

F32 = mybir.dt.float32
BF16 = mybir.dt.bfloat16

DT_Y = BF16    # y8 replicas
DT_ACC = BF16  # pacc accumulators / products

H = W = 128
CIN = 8
COUT = 32
K = 3
OH = OW = 126
NPIX = H * OH          # 16128 flat pixels (h*128+w), h<126
NCORES = 8
# conv free-dim chunks: two 512-col leading chunks cover band 1 exactly
_CONV_SIZES = [512] * 2 + [384] * 39 + [128]
CONV_CHUNKS = []
_n0 = 0
for _s in _CONV_SIZES:
    CONV_CHUNKS.append((_n0, _s))
    _n0 += _s
assert _n0 == NPIX
NCHUNK = len(CONV_CHUNKS)
CHUNK_MAX = max(_CONV_SIZES)
GROUP = 8              # reduce-phase chunks (=output rows) per psum group
BANDS = [(0, 8), (8, 40), (40, 72), (72, 104), (104, 126)]
BH_MAX = max(b - a for a, b in BANDS)

TAPS = [(kh, kw) for kh in range(K) for kw in range(K)]
# products computed on DVE (tensor_scalar 4x); the rest on ACT (mul).
VTAPS = {(1, 0), (1, 1), (2, 0), (2, 1), (3, 0), (3, 1)}


def _r3(t, h0, nh, w0, nw):
    """3D region view [128, nh, nw] of a [128, NPIX] tile at rows h0, cols w0."""
    return t[:].rearrange("p (h w) -> p h w", w=W)[:, h0 : h0 + nh, w0 : w0 + nw]


def build_program():
    nc = bacc.Bacc()

    # x72: host-built im2col, x72[(kh*3+kw)*8+ci, pix] = x[ci, pix+kh*128+kw]
    x_d = nc.declare_dram_parameter("x72", [72, NPIX], BF16, isOutput=False)
    # consts: [:, 0:18] scal, [:, 18:19] bias128, [0:72, 19:83] w1rep (bf16 pairs)
    c_d = nc.declare_dram_parameter("consts", [128, 83], F32, isOutput=False)
    out_d = nc.declare_dram_parameter("out", [OH, OW], F32, isOutput=True)

    with ExitStack() as ctx:
        tc = ctx.enter_context(tile.TileContext(nc))

        consts = ctx.enter_context(tc.tile_pool(name="consts", bufs=1))
        big = ctx.enter_context(tc.tile_pool(name="big", bufs=1))

        constst = consts.tile([128, 83], F32)
        # issue from the scalar queue so it overlaps the x72 DMA issue below
        nc.scalar.dma_start(constst[:], c_d[:])
        scalt = constst[:, 0:18]
        biast = constst[:, 18:19]
        w1t = constst[0:72, 19:83].bitcast(BF16)  # [72, 128] bf16
        ident = consts.tile([128, 128], DT_ACC)
        masks.make_identity(nc, ident[:])

        # y8 padded with one junk row so contiguous kw-shifted reads stay
        # in-bounds; pad is zeroed to avoid NaN garbage.
        y8 = big.tile([128, NPIX + W], DT_Y)
        nc.vector.memset(y8[:, NPIX : NPIX + W], 0.0)
        # both co-halves in one tile: half s at cols [s*NPIX, (s+1)*NPIX)
        pacc2 = big.tile([128, 2 * NPIX], DT_ACC)
        outt = big.tile([128, OH], DT_ACC)  # OUT[w, h]

        ppool = ctx.enter_context(tc.tile_pool(name="ppool", bufs=8))
        redpool = ctx.enter_context(tc.tile_pool(name="redpool", bufs=2))
        respool = ctx.enter_context(tc.tile_pool(name="respool", bufs=2))
        xp = ctx.enter_context(tc.tile_pool(name="xp", bufs=4))
        psum = ctx.enter_context(tc.tile_pool(name="psum", bufs=3, space="PSUM"))
        psred = ctx.enter_context(tc.tile_pool(name="psred", bufs=2, space="PSUM"))
        psout = ctx.enter_context(tc.tile_pool(name="psout", bufs=1, space="PSUM"))

        def conv_chunks(c_lo, c_hi):
            for c in range(c_lo, c_hi):
                n0, sz = CONV_CHUNKS[c]
                xh = xp.tile([72, CHUNK_MAX], BF16, tag="xh")
                nc.sync.dma_start(out=xh[:, 0:sz], in_=x_d[:, n0 : n0 + sz])
                ps = psum.tile([128, CHUNK_MAX], F32, tag="convps")
                nc.tensor.matmul(
                    ps[:, 0:sz], lhsT=w1t, rhs=xh[:, 0:sz], start=True, stop=True)
                nc.scalar.activation(
                    y8[:, n0 : n0 + sz], ps[:, 0:sz],
                    mybir.ActivationFunctionType.Identity,
                    bias=biast, scale=1.0,
                )

        mx = mybir.AluOpType.max
        mn = mybir.AluOpType.min

        def reduce_group(c0, gc):
            """Fold chunks [c0, c0+gc) of both pacc halves into outt[:, c0:c0+gc]."""
            ps_ab = psred.tile([128, 2 * GROUP * 128], DT_ACC, tag="ps_ab")
            for half in range(2):
                for j in range(gc):
                    nc.tensor.transpose(
                        ps_ab[:, (half * gc + j) * 128 : (half * gc + j + 1) * 128],
                        pacc2[:, half * NPIX + (c0 + j) * 128
                              : half * NPIX + (c0 + j + 1) * 128],
                        ident[:],
                    )
            pt = redpool.tile([128, 2 * GROUP * 128], DT_ACC, tag="PT")
            nc.scalar.copy(pt[:, 0 : 2 * gc * 128], ps_ab[:, 0 : 2 * gc * 128])
            eng = nc.vector
            # pt layout: [p][s=2][c=gc][ci=8][co=16] (valid region only)
            v = pt[:, 0 : 2 * gc * 128].rearrange(
                "p (s c ci co) -> p s c ci co", s=2, c=gc, ci=8
            )
            eng.tensor_tensor(
                v[:, :, :, 0:4, :], v[:, :, :, 0:4, :], v[:, :, :, 4:8, :], mx)
            eng.tensor_tensor(
                v[:, :, :, 0:2, :], v[:, :, :, 0:2, :], v[:, :, :, 2:4, :], mx)
            eng.tensor_tensor(
                v[:, :, :, 0:1, :], v[:, :, :, 0:1, :], v[:, :, :, 1:2, :], mx)
            # min over both halves and all 16 co in one 2-axis reduce
            w2 = pt[:, 0 : 2 * gc * 128].rearrange(
                "p (s c ci co) -> p c ci s co", s=2, c=gc, ci=8
            )[:, :, 0, :, :]  # [p, c, s, co]
            eng.tensor_reduce(
                outt[:, c0 : c0 + gc].rearrange("p (c a b) -> p c a b", a=1, b=1),
                w2, mybir.AxisListType.XY, mn)

        def band_groups(h0, h1):
            return [(c0, min(GROUP, h1 - c0)) for c0 in range(h0, h1, GROUP)]

        def out_slice(h0, h1):
            """Transpose finished outt[:, h0:h1] -> rows, convert, DMA out."""
            n = h1 - h0
            pso = psout.tile([128, 128], DT_ACC, tag="pso")
            nc.tensor.transpose(pso[0:n, :], outt[:, h0:h1], ident[:])
            res = respool.tile([128, 128], F32, tag="res")
            nc.scalar.copy(res[0:n, :], pso[0:n, :])
            nc.sync.dma_start(out_d[h0:h1, :], res[0:n, 0:OW])

        # conv rows 0..10 upfront: covers band-1 taps incl. kh=2 lookahead
        conv_chunks(0, 3)

        # --- step 2, banded; fold band k-1 spread across band k's taps ---
        for bi, (h0, h1) in enumerate(BANDS):
            bh = h1 - h0
            prev_groups = band_groups(*BANDS[bi - 1]) if bi > 0 else []
            nfold = len(prev_groups)
            # tap 0 covers the full band (incl. junk cols 126/127)
            for half in range(2):
                nc.vector.tensor_scalar(
                    pacc2[:, half * NPIX + h0 * W : half * NPIX + h1 * W],
                    y8[:, h0 * W : h1 * W],
                    scalt[:, half * 9 : half * 9 + 1], None, mybir.AluOpType.mult,
                )

            pending = {}

            def emit_products(t):
                kh, kw = TAPS[t]
                nh = min(h1, OH - kh) - h0
                nw = OW - kw
                prods = []
                prods = []
                for half in range(2):
                    p = ppool.tile([128, BH_MAX * W], DT_ACC, tag="P")
                    # contiguous full-width product (junk cols >= 126-kw are
                    # never read by the max below); keeps ACT in its fast mode
                    src = y8[:, (h0 + kh) * W + kw : (h0 + kh) * W + kw + nh * W]
                    sc = scalt[:, half * 9 + t : half * 9 + t + 1]
                    if (t, half) in VTAPS:
                        nc.vector.tensor_scalar(
                            p[:, 0 : nh * W], src, sc, None, mybir.AluOpType.mult)
                    else:
                        nc.scalar.mul(p[:, 0 : nh * W], src, sc)
                    p3 = p[:].rearrange("p (h w) -> p h w", w=W)[:, 0:nh, 0:nw]
                    prods.append(p3)
                pending[t] = (nh, nw, prods)

            emit_products(1)
            emit_products(2)
            emit_products(3)
            emit_products(4)
            for t in range(1, 9):
                nh, nw, prods = pending.pop(t)
                for half in range(2):
                    acc3 = pacc2[:, half * NPIX :].rearrange(
                        "p (h w) -> p h w", w=W)[:, h0 : h0 + nh, 0:nw]
                    nc.vector.tensor_tensor(acc3, acc3, prods[half], mx)
                if t <= 4:
                    emit_products(t + 4)
                # rest of conv rides behind bands 1-3, interleaved finely in
                # band 1 so its ACT products are not starved
                if bi == 0 and 1 <= t <= 3:
                    conv_chunks(3 + (t - 1) * 5, 3 + t * 5)
                if bi == 1 and t == 1:
                    conv_chunks(18, 30)
                if bi == 2 and t == 1:
                    conv_chunks(30, NCHUNK)
                # fold groups of the previous band spread across taps 2..7
                if prev_groups and 2 <= t < 2 + nfold:
                    reduce_group(*prev_groups.pop(0))
            while prev_groups:
                reduce_group(*prev_groups.pop(0))
            if bi > 0:
                out_slice(*BANDS[bi - 1])
        for g in band_groups(*BANDS[-1]):
            reduce_group(*g)
        out_slice(*BANDS[-1])

    nc.compile()
    return nc


def host_tiles(weight, bias):
    weight = np.asarray(weight, np.float32)
    bias = np.asarray(bias, np.float32)
    w1rep = np.zeros((72, 128), np.float32)
    for kh in range(K):
        for kw in range(K):
            for ci_in in range(CIN):
                t = (kh * K + kw) * CIN + ci_in
                for ci_out in range(CIN):
                    w1rep[t, ci_out * 16 : ci_out * 16 + 16] = weight[
                        ci_out, ci_in, kh, kw
                    ]
    bias128 = np.repeat(bias[:CIN], 16).astype(np.float32).reshape(128, 1)
    scal = np.zeros((128, 18), np.float32)
    for p in range(128):
        ci = p // 16
        co_lo = p % 16
        for half in range(2):
            co = co_lo + 16 * half
            for t, (kh, kw) in enumerate(TAPS):
                scal[p, half * 9 + t] = 2.0 * weight[co, ci, kh, kw]
    consts = np.zeros((128, 83), np.float32)
    consts[:, 0:18] = scal
    consts[:, 18:19] = bias128
    w1b = w1rep.astype(BF16_NP)  # [72, 128] -> bitcast into f32 cols 19:83
    consts[0:72, 19:83] = w1b.view(np.uint16).reshape(72, 64, 2).view(
        np.uint32).reshape(72, 64).view(np.float32)
    return consts


def im2col_host(xb):
    """xb: (8,128,128) f32 -> (72, NPIX) bf16 with junk tail cols zeroed."""
    x72 = np.zeros((72, NPIX), np.float32)
    L = NPIX - 2
    flat = xb.reshape(-1)
    for kh in range(K):
        for kw in range(K):
            for ci in range(CIN):
                t = (kh * K + kw) * CIN + ci
                off = kh * W + kw
                x72[t, :L] = flat[ci * H * W + off : ci * H * W + off + L]
    return x72.astype(BF16_NP)


_CACHE = {}


def _get_program():
    if "nc" not in _CACHE:
        _CACHE["nc"] = build_program()
    return _CACHE["nc"]


def run_spmd(x, weight, bias, **kw):
    x = np.ascontiguousarray(np.asarray(x, np.float32))
    consts = host_tiles(weight, bias)
    nc = _get_program()
    in_maps = [
        {"x72": im2col_host(x[b]), "consts": consts} for b in range(NCORES)
    ]
    res = run_bass_kernel_spmd(nc, in_maps, list(range(NCORES)), **kw)
    out = np.stack([res.results[b]["out"] for b in range(NCORES)])
    return out[:, None, :, :].astype(np.float32), res


def kernel(x, weight, bias):
    out, _ = run_spmd(x, weight, bias)
    return out


if __name__ == "__main__":
    rng = np.random.default_rng(0)
    x = rng.standard_normal((8, CIN, H, W), dtype=np.float32)
    wt = rng.uniform(-0.1, 0.1, (COUT, CIN, K, K)).astype(np.float32)
    bs = rng.uniform(-0.1, 0.1, COUT).astype(np.float32)
    print(kernel(x, wt, bs).shape)


# revision 66
# speedup vs baseline: 1.0131x; 1.0131x over previous
"""Trainium2 Bass kernel for nn_ModelNew_3556232922055 (dense_cnn).

Semantics (per image):
  y8[j]    = conv2d_valid(x, weight[:8]) + bias[:8]          (8,126,126)
  acc[co]  = max over (ci,kh,kw) of 2*W[co,ci,kh,kw]*y8[ci,h+kh,w+kw]
             (out-of-range taps excluded at the bottom/right borders)
  out      = min over co of acc                              (1,126,126)

Sharding: data-parallel over batch, 1 image per NeuronCore (8 cores).

Device mapping per core:
  - host-built im2col X72 [72, 16128] bf16, streamed per conv chunk
  - conv as k=72 bf16 matmuls -> PSUM -> ACT evac (+bias) into
    Y8REP [128, 16128] bf16 where partition p = ci*16 + r holds y8[ci]
  - step 2 in row bands; per band, per tap (kh,kw), per co-half
    (A: co 0-15 on partition ci*16+co, B: co 16-31):
    product = scal[p,t]*y8[ci, pix+off] on DVE (4x ts) or ACT (mul),
    then DVE tensor_tensor(max) into pacc[half]
  - fold of band k emitted during band k+1: PE-transpose 128x128
    chunks -> ACT evac -> DVE max-tree over ci, min over 32 co
    -> OUT[w,h]; finally PE transpose -> DMA out (126,126) f32.
"""

import numpy as np
from contextlib import ExitStack

import concourse.bass as bass
import concourse.tile as tile
from concourse import bacc, mybir
from concourse import masks
from concourse.bass_utils import run_bass_kernel_spmd

try:
    import ml_dtypes
    BF16_NP = ml_dtypes.bfloat16
except ImportError:  # pragma: no cover
    import jax.numpy as jnp
    BF16_NP = jnp.bfloat16

F32 = mybir.dt.float32
BF16 = mybir.dt.bfloat16

DT_Y = BF16    # y8 replicas
DT_ACC = BF16  # pacc accumulators / products

H = W = 128
CIN = 8
COUT = 32
K = 3
OH = OW = 126
NPIX = H * OH          # 16128 flat pixels (h*128+w), h<126
NCORES = 8
# conv free-dim chunks: two 512-col leading chunks cover band 1 exactly
_CONV_SIZES = [512] * 2 + [384] * 39 + [128]
CONV_CHUNKS = []
_n0 = 0
for _s in _CONV_SIZES:
    CONV_CHUNKS.append((_n0, _s))
    _n0 += _s
assert _n0 == NPIX
NCHUNK = len(CONV_CHUNKS)
CHUNK_MAX = max(_CONV_SIZES)
GROUP = 8              # reduce-phase chunks (=output rows) per psum group
BANDS = [(0, 8), (8, 40), (40, 72), (72, 104), (104, 126)]
BH_MAX = max(b - a for a, b in BANDS)

TAPS = [(kh, kw) for kh in range(K) for kw in range(K)]
# products computed on DVE (tensor_scalar 4x); the rest on ACT (mul).
VTAPS = {(1, 0), (1, 1), (2, 0), (2, 1), (3, 0), (3, 1)}


def _r3(t, h0, nh, w0, nw):
    """3D region view [128, nh, nw] of a [128, NPIX] tile at rows h0, cols w0."""
    return t[:].rearrange("p (h w) -> p h w", w=W)[:, h0 : h0 + nh, w0 : w0 + nw]


def build_program():
    nc = bacc.Bacc()

    # x72: host-built im2col, x72[(kh*3+kw)*8+ci, pix] = x[ci, pix+kh*128+kw]
    x_d = nc.declare_dram_parameter("x72", [72, NPIX], BF16, isOutput=False)
    # consts: [:, 0:18] scal, [:, 18:19] bias128, [0:72, 19:83] w1rep (bf16 pairs)
    c_d = nc.declare_dram_parameter("consts", [128, 83], F32, isOutput=False)
    out_d = nc.declare_dram_parameter("out", [OH, OW], F32, isOutput=True)

    with ExitStack() as ctx:
        tc = ctx.enter_context(tile.TileContext(nc))

        consts = ctx.enter_context(tc.tile_pool(name="consts", bufs=1))
        big = ctx.enter_context(tc.tile_pool(name="big", bufs=1))

        constst = consts.tile([128, 83], F32)
        # issue from the scalar queue so it overlaps the x72 DMA issue below
        nc.scalar.dma_start(constst[:], c_d[:])
        scalt = constst[:, 0:18]
        biast = constst[:, 18:19]
        w1t = constst[0:72, 19:83].bitcast(BF16)  # [72, 128] bf16
        ident = consts.tile([128, 128], DT_ACC)
        masks.make_identity(nc, ident[:])

        # y8 padded with one junk row so contiguous kw-shifted reads stay
        # in-bounds; pad is zeroed to avoid NaN garbage.
        y8 = big.tile([128, NPIX + W], DT_Y)
        nc.vector.memset(y8[:, NPIX : NPIX + W], 0.0)
        # both co-halves in one tile: half s at cols [s*NPIX, (s+1)*NPIX)
        pacc2 = big.tile([128, 2 * NPIX], DT_ACC)
        outt = big.tile([128, OH], DT_ACC)  # OUT[w, h]

        ppool = ctx.enter_context(tc.tile_pool(name="ppool", bufs=8))
        redpool = ctx.enter_context(tc.tile_pool(name="redpool", bufs=2))
        respool = ctx.enter_context(tc.tile_pool(name="respool", bufs=2))
        xp = ctx.enter_context(tc.tile_pool(name="xp", bufs=4))
        psum = ctx.enter_context(tc.tile_pool(name="psum", bufs=3, space="PSUM"))
        psred = ctx.enter_context(tc.tile_pool(name="psred", bufs=2, space="PSUM"))
        psout = ctx.enter_context(tc.tile_pool(name="psout", bufs=1, space="PSUM"))

        def conv_chunks(c_lo, c_hi):
            for c in range(c_lo, c_hi):
                n0, sz = CONV_CHUNKS[c]
                xh = xp.tile([72, CHUNK_MAX], BF16, tag="xh")
                nc.sync.dma_start(out=xh[:, 0:sz], in_=x_d[:, n0 : n0 + sz])
                ps = psum.tile([128, CHUNK_MAX], F32, tag="convps")
                nc.tensor.matmul(
                    ps[:, 0:sz], lhsT=w1t, rhs=xh[:, 0:sz], start=True, stop=True)
                nc.scalar.activation(
                    y8[:, n0 : n0 + sz], ps[:, 0:sz],
                    mybir.ActivationFunctionType.Identity,
                    bias=biast, scale=1.0,
                )

        mx = mybir.AluOpType.max
        mn = mybir.AluOpType.min

        def reduce_group(c0, gc):
            """Fold chunks [c0, c0+gc) of both pacc halves into outt[:, c0:c0+gc]."""
            ps_ab = psred.tile([128, 2 * GROUP * 128], DT_ACC, tag="ps_ab")
            for half in range(2):
                for j in range(gc):
                    nc.tensor.transpose(
                        ps_ab[:, (half * gc + j) * 128 : (half * gc + j + 1) * 128],
                        pacc2[:, half * NPIX + (c0 + j) * 128
                              : half * NPIX + (c0 + j + 1) * 128],
                        ident[:],
                    )
            pt = redpool.tile([128, 2 * GROUP * 128], DT_ACC, tag="PT")
            nc.scalar.copy(pt[:, 0 : 2 * gc * 128], ps_ab[:, 0 : 2 * gc * 128])
            eng = nc.vector
            # pt layout: [p][s=2][c=gc][ci=8][co=16] (valid region only)
            v = pt[:, 0 : 2 * gc * 128].rearrange(
                "p (s c ci co) -> p s c ci co", s=2, c=gc, ci=8
            )
            eng.tensor_tensor(
                v[:, :, :, 0:4, :], v[:, :, :, 0:4, :], v[:, :, :, 4:8, :], mx)
            eng.tensor_tensor(
                v[:, :, :, 0:2, :], v[:, :, :, 0:2, :], v[:, :, :, 2:4, :], mx)
            eng.tensor_tensor(
                v[:, :, :, 0:1, :], v[:, :, :, 0:1, :], v[:, :, :, 1:2, :], mx)
            # min over both halves and all 16 co in one 2-axis reduce
            w2 = pt[:, 0 : 2 * gc * 128].rearrange(
                "p (s c ci co) -> p c ci s co", s=2, c=gc, ci=8
            )[:, :, 0, :, :]  # [p, c, s, co]
            eng.tensor_reduce(
                outt[:, c0 : c0 + gc].rearrange("p (c a b) -> p c a b", a=1, b=1),
                w2, mybir.AxisListType.XY, mn)

        def band_groups(h0, h1):
            return [(c0, min(GROUP, h1 - c0)) for c0 in range(h0, h1, GROUP)]

        def out_slice(h0, h1):
            """Transpose finished outt[:, h0:h1] -> rows, convert, DMA out."""
            n = h1 - h0
            pso = psout.tile([128, 128], DT_ACC, tag="pso")
            nc.tensor.transpose(pso[0:n, :], outt[:, h0:h1], ident[:])
            res = respool.tile([128, 128], F32, tag="res")
            nc.scalar.copy(res[0:n, :], pso[0:n, :])
            nc.sync.dma_start(out_d[h0:h1, :], res[0:n, 0:OW])

        # conv rows 0..10 upfront: covers band-1 taps incl. kh=2 lookahead
        conv_chunks(0, 3)

        # --- step 2, banded; fold band k-1 spread across band k's taps ---
        for bi, (h0, h1) in enumerate(BANDS):
            bh = h1 - h0
            prev_groups = band_groups(*BANDS[bi - 1]) if bi > 0 else []
            nfold = len(prev_groups)
            # tap 0 covers the full band (incl. junk cols 126/127)
            for half in range(2):
                nc.vector.tensor_scalar(
                    pacc2[:, half * NPIX + h0 * W : half * NPIX + h1 * W],
                    y8[:, h0 * W : h1 * W],
                    scalt[:, half * 9 : half * 9 + 1], None, mybir.AluOpType.mult,
                )

            pending = {}

            def emit_products(t):
                kh, kw = TAPS[t]
                nh = min(h1, OH - kh) - h0
                nw = OW - kw
                prods = []
                prods = []
                for half in range(2):
                    p = ppool.tile([128, BH_MAX * W], DT_ACC, tag="P")
                    # contiguous full-width product (junk cols >= 126-kw are
                    # never read by the max below); keeps ACT in its fast mode
                    src = y8[:, (h0 + kh) * W + kw : (h0 + kh) * W + kw + nh * W]
                    sc = scalt[:, half * 9 + t : half * 9 + t + 1]
                    if (t, half) in VTAPS:
                        nc.vector.tensor_scalar(
                            p[:, 0 : nh * W], src, sc, None, mybir.AluOpType.mult)
                    else:
                        nc.scalar.mul(p[:, 0 : nh * W], src, sc)
                    p3 = p[:].rearrange("p (h w) -> p h w", w=W)[:, 0:nh, 0:nw]
                    prods.append(p3)
                pending[t] = (nh, nw, prods)

            emit_products(1)
            emit_products(2)
            emit_products(3)
            emit_products(4)
            for t in range(1, 9):
                nh, nw, prods = pending.pop(t)
                for half in range(2):
                    acc3 = pacc2[:, half * NPIX :].rearrange(
                        "p (h w) -> p h w", w=W)[:, h0 : h0 + nh, 0:nw]
                    nc.vector.tensor_tensor(acc3, acc3, prods[half], mx)
                if t <= 4:
                    emit_products(t + 4)
                # rest of conv rides behind bands 1-3, interleaved finely in
                # band 1 so its ACT products are not starved
                if bi == 0 and 1 <= t <= 5:
                    conv_chunks(3 + (t - 1) * 3, 3 + t * 3)
                if bi == 1 and t == 1:
                    conv_chunks(18, 30)
                if bi == 2 and t == 1:
                    conv_chunks(30, NCHUNK)
                # fold groups of the previous band spread across taps 2..7
                if prev_groups and 2 <= t < 2 + nfold:
                    reduce_group(*prev_groups.pop(0))
            while prev_groups:
                reduce_group(*prev_groups.pop(0))
            if bi > 0:
                out_slice(*BANDS[bi - 1])
        for g in band_groups(*BANDS[-1]):
            reduce_group(*g)
        out_slice(*BANDS[-1])

    nc.compile()
    return nc


def host_tiles(weight, bias):
    weight = np.asarray(weight, np.float32)
    bias = np.asarray(bias, np.float32)
    w1rep = np.zeros((72, 128), np.float32)
    for kh in range(K):
        for kw in range(K):
            for ci_in in range(CIN):
                t = (kh * K + kw) * CIN + ci_in
                for ci_out in range(CIN):
                    w1rep[t, ci_out * 16 : ci_out * 16 + 16] = weight[
                        ci_out, ci_in, kh, kw
                    ]
    bias128 = np.repeat(bias[:CIN], 16).astype(np.float32).reshape(128, 1)
    scal = np.zeros((128, 18), np.float32)
    for p in range(128):
        ci = p // 16
        co_lo = p % 16
        for half in range(2):
            co = co_lo + 16 * half
            for t, (kh, kw) in enumerate(TAPS):
                scal[p, half * 9 + t] = 2.0 * weight[co, ci, kh, kw]
    consts = np.zeros((128, 83), np.float32)
    consts[:, 0:18] = scal
    consts[:, 18:19] = bias128
    w1b = w1rep.astype(BF16_NP)  # [72, 128] -> bitcast into f32 cols 19:83
    consts[0:72, 19:83] = w1b.view(np.uint16).reshape(72, 64, 2).view(
        np.uint32).reshape(72, 64).view(np.float32)
    return consts


def im2col_host(xb):
    """xb: (8,128,128) f32 -> (72, NPIX) bf16 with junk tail cols zeroed."""
    x72 = np.zeros((72, NPIX), np.float32)
    L = NPIX - 2
    flat = xb.reshape(-1)
    for kh in range(K):
        for kw in range(K):
            for ci in range(CIN):
                t = (kh * K + kw) * CIN + ci
                off = kh * W + kw
                x72[t, :L] = flat[ci * H * W + off : ci * H * W + off + L]
    return x72.astype(BF16_NP)


_CACHE = {}


def _get_program():
    if "nc" not in _CACHE:
        _CACHE["nc"] = build_program()
    return _CACHE["nc"]


def run_spmd(x, weight, bias, **kw):
    x = np.ascontiguousarray(np.asarray(x, np.float32))
    consts = host_tiles(weight, bias)
    nc = _get_program()
    in_maps = [
        {"x72": im2col_host(x[b]), "consts": consts} for b in range(NCORES)
    ]
    res = run_bass_kernel_spmd(nc, in_maps, list(range(NCORES)), **kw)
    out = np.stack([res.results[b]["out"] for b in range(NCORES)])
    return out[:, None, :, :].astype(np.float32), res


def kernel(x, weight, bias):
    out, _ = run_spmd(x, weight, bias)
    return out


if __name__ == "__main__":
    rng = np.random.default_rng(0)
    x = rng.standard_normal((8, CIN, H, W), dtype=np.float32)
    wt = rng.uniform(-0.1, 0.1, (COUT, CIN, K, K)).astype(np.float32)
    bs = rng.uniform(-0.1, 0.1, COUT).astype(np.float32)
    print(kernel(x, wt, bs).shape)
